# revision 18
# baseline (speedup 1.0000x reference)
"""Bahdanau-attention kernel for Trainium2 (8 NeuronCores, Bass/Tile).

Computation (reference, fp32):
    Wh  = hidden @ W_w.T + W_b                      # [B, H]
    Ue  = einsum('bse,he->bsh', enc^T, U_w) + U_b   # [B, S, H]
    en  = tanh(Wh[:,None,:] + Ue) @ v_w[0]          # [B, S]
    out = softmax(where(mask, -1e10, en), axis=1)

Strategy (v2 - fp8 DoubleRow):
- Data-parallel over batch: 8 rows per core. Rows are rank-sorted by
  unmasked count so position p on every core is padded to the same
  compile-time width w[p]; only unmasked s-columns are packed (exact:
  exp(-1e10) = 0).
- Main GEMM in fp8 e4m3 with perf_mode=DoubleRow: each matmul contracts
  256 rows (two 128-chunks), halving PE streaming time vs bf16. U is
  scaled by 64 before quantization; the tanh activation un-scales.
- Positions are bin-packed into column tiles <= 512 (one PSUM bank
  each). Loop order per h-chunk is ec2-major: one weight pair feeds all
  bins back-to-back so LDWEIGHTS overlaps the matmul stream.
- Wh (bf16) runs as small interleaved chains; W_b+U_b folded into the
  per-partition tanh bias.
- v-weighting runs on the Vector engine (tmp = tanh*v in bf16; acc +=
  tmp in fp32; the last h-chunk's add writes a bf16 copy and the -1e10
  mask row is added into partition 127); final partition reduce is one
  bf16 ones-matmul per position into PSUM strips (partition 32*(i%4)).
- Softmax: one bank-wide Exp (+per-partition accum) per group of 4
  positions, reciprocal, one broadcast multiply, one 4-row output DMA.
- DMA: both HWDGE queues (sync + scalar) stream enc/uw with e0/uw0
  first; gpsimd SWDGE carries the later-needed ww/uw chunks.
"""

import numpy as np
import ml_dtypes

B, S, H, E = 64, 512, 1024, 2048
NCORES = 8
BL = B // NCORES          # rows (positions) per core
HC = H // 128             # h chunks
EC2 = E // 256            # fp8 DoubleRow chunk pairs
KC = H // 128             # k chunks for the Wh matmul
NEG = np.float32(-1e10)
U_SCALE = 64.0

bf16 = ml_dtypes.bfloat16
fp8 = ml_dtypes.float8_e4m3

_CACHE = {}


def _plan(widths):
    """Bin-pack position widths into column tiles <= 512 (first-fit dec.).

    Returns (placements, bins) where placements is a list of original
    position indices in column order and bins is a list of
    (bin_off, bin_w, [(placement_idx, local_off, w), ...]).
    """
    order = sorted(range(len(widths)), key=lambda p: -widths[p])
    bins = []  # list of lists of original position idx
    for p in order:
        for bn in bins:
            if sum(widths[q] for q in bn) + widths[p] <= 512:
                bn.append(p)
                break
        else:
            bins.append([p])
    placements = [p for bn in bins for p in bn]
    out_bins = []
    off = 0
    pi = 0
    for bn in bins:
        lo = 0
        segs = []
        for p in bn:
            segs.append((pi, lo, widths[p]))
            lo += widths[p]
            pi += 1
        out_bins.append((off, lo, segs))
        off += lo
    return placements, out_bins


def _build_nc(widths):
    """Per-core program; widths = per-position packed col counts (<=512)."""
    import concourse.mybir as mybir
    import concourse.tile as tile
    from concourse import bacc

    F32 = mybir.dt.float32
    F32R = mybir.dt.float32r
    BF = mybir.dt.bfloat16
    F8 = mybir.dt.float8e4
    AF = mybir.ActivationFunctionType
    DR = mybir.MatmulPerfMode.DoubleRow

    placements, bins = _plan(widths)
    NPOS = len(widths)
    X = sum(widths)
    XP = -(-X // 16) * 16           # plane stride must be 16B-aligned
    NB = len(bins)
    assert NB + 3 <= 8, f"too many PSUM banks: {NB}"
    # placement -> (col_off, width)
    pl_off = [None] * NPOS
    for boff, bw, segs in bins:
        for pi, lo, w in segs:
            pl_off[pi] = (boff + lo, w)

    # per-bin 16-padded plane stride for the fp8 DoubleRow rhs
    bw16 = [-(-bw // 16) * 16 for (_, bw, _) in bins]
    boff16 = [0]
    for b in bw16:
        boff16.append(boff16[-1] + 2 * b)
    SUMW = boff16[-1]

    nc = bacc.Bacc(num_swdge_queues=4)
    enc8 = nc.declare_dram_parameter("enc8", [EC2, 128, SUMW], F8, isOutput=False)
    uw8 = nc.declare_dram_parameter("uw8", [HC, 128, EC2 * 2 * 128], F8, isOutput=False)
    wwT = nc.declare_dram_parameter("wwT", [HC, 128, KC * 128], BF, isOutput=False)
    hidT = nc.declare_dram_parameter("hidT", [128, KC * BL], BF, isOutput=False)
    vt = nc.declare_dram_parameter("vt", [128, HC], F32, isOutput=False)
    bc = nc.declare_dram_parameter("bc", [128, HC], F32, isOutput=False)
    amask = nc.declare_dram_parameter("amask", [1, XP], BF, isOutput=False)
    NG = (NPOS + 3) // 4
    corr = nc.declare_dram_parameter("corr", [128, NG], F32, isOutput=False)
    out_d = nc.declare_dram_parameter("out", [NPOS, 512], F32, isOutput=True)

    with tile.TileContext(nc) as tc:
        with (
            tc.tile_pool(name="const", bufs=1) as cst,
            tc.tile_pool(name="wpool", bufs=1) as wp,
            tc.tile_pool(name="encp", bufs=EC2) as encp,
            tc.tile_pool(name="thp", bufs=4) as thp,
            tc.tile_pool(name="tmpp", bufs=3) as tmpp,
            tc.tile_pool(name="pwp", bufs=1, space="PSUM") as pwp,
            tc.tile_pool(name="pup", bufs=NB, space="PSUM") as pup,
            tc.tile_pool(name="pep", bufs=1, space="PSUM") as pep,
        ):
            # ---- DMAs: sync + scalar are HWDGE queues (fast); gpsimd is
            # software DGE and carries only later-needed weights/constants.
            hid_sb = cst.tile([128, KC * BL], BF, tag="hid")
            nc.sync.dma_start(hid_sb[:], hidT[:])
            bc_sb = cst.tile([128, HC], F32, tag="bc")
            nc.gpsimd.dma_start(bc_sb[:], bc[:])
            vt_sb = cst.tile([128, HC], F32, tag="vt")
            nc.gpsimd.dma_start(vt_sb[:], vt[:])
            am_sb = cst.tile([1, XP], BF, tag="am")
            nc.gpsimd.dma_start(am_sb[:], amask[:])

            ww_sb = [wp.tile([128, KC * 128], BF, tag=f"ww{h}", name=f"ww{h}") for h in range(HC)]
            uw_sb = [wp.tile([128, EC2 * 256], F8, tag=f"uw{h}", name=f"uw{h}") for h in range(HC)]
            enc_sb = [encp.tile([128, SUMW], F8, tag="enc", name=f"enc{k}") for k in range(EC2)]

            def dma2(dst, src, width, e1, e2):
                h = (width // 2) & ~3
                e1.dma_start(dst[:, 0:h], src[:, 0:h])
                e2.dma_start(dst[:, h:width], src[:, h:width])

            # startup-critical: small chunks so the Wh chain and first main
            # matmuls can start as data lands; 3-way split incl. gpsimd
            for c in range(4):
                eng = (nc.sync, nc.scalar)[c % 2]
                eng.dma_start(ww_sb[0][:, c * 256:(c + 1) * 256],
                              wwT[0][:, c * 256:(c + 1) * 256])
            third = (EC2 * 256 // 3) & ~15
            nc.sync.dma_start(uw_sb[0][:, 0:third], uw8[0][:, 0:third])
            nc.scalar.dma_start(uw_sb[0][:, third:2 * third], uw8[0][:, third:2 * third])
            nc.gpsimd.dma_start(uw_sb[0][:, 2 * third:], uw8[0][:, 2 * third:])
            # e0 in 3-way fine chunks for earliest start; e1..e5 as half-tile
            # transfers (DMA triggers cost ~0.7us each on the queue engine)
            qi = 0
            for t in range(NB):
                lo, hi = boff16[t], boff16[t + 1]
                eng = (nc.sync, nc.scalar, nc.gpsimd)[qi % 3]
                qi += 1
                eng.dma_start(enc_sb[0][:, lo:hi], enc8[0][:, lo:hi])
            half = (SUMW // 2) & ~15
            for k in range(1, 6):
                nc.sync.dma_start(enc_sb[k][:, 0:half], enc8[k][:, 0:half])
                nc.scalar.dma_start(enc_sb[k][:, half:SUMW], enc8[k][:, half:SUMW])
                if k == 2:
                    nc.scalar.dma_start(uw_sb[1][:], uw8[1][:])
                if k == 4:
                    nc.sync.dma_start(uw_sb[2][:], uw8[2][:])
                if k == 5:
                    nc.scalar.dma_start(uw_sb[3][:], uw8[3][:])
            # e6/e7 + later weights ride the SWDGE queue, relieving the
            # HWDGE pair during the startup-critical window
            nc.gpsimd.dma_start(ww_sb[1][:], wwT[1][:])
            for k in (6, 7):
                h = (SUMW // 2) & ~15
                nc.gpsimd.dma_start(enc_sb[k][:, 0:h], enc8[k][:, 0:h])
                nc.gpsimd.dma_start(enc_sb[k][:, h:SUMW], enc8[k][:, h:SUMW])
            for k in range(2, HC):
                nc.gpsimd.dma_start(ww_sb[k][:], wwT[k][:])
            for h in range(4, HC):
                nc.gpsimd.dma_start(uw_sb[h][:], uw8[h][:])

            bias_sb = cst.tile([128, HC * BL], F32, tag="bias")
            vtb_sb = cst.tile([128, HC], BF, tag="vtb")
            nc.vector.tensor_copy(vtb_sb[:], vt_sb[:])
            acc = cst.tile([128, XP], F32, tag="acc")
            accb = cst.tile([128, XP], BF, tag="accb")
            ones_c = cst.tile([128, 1], BF, tag="onesc")
            nc.vector.memset(ones_c[:], 1.0)

            corr_sb = cst.tile([128, NG], F32, tag="corr")
            nc.gpsimd.dma_start(corr_sb[:], corr[:])
            res_g = [cst.tile([128, 512], F32, tag=f"res{g}", name=f"res{g}") for g in range(NG)]
            ss_g = [cst.tile([128, 1], F32, tag=f"ss{g}", name=f"ss{g}") for g in range(NG)]
            rcp_g = [cst.tile([128, 1], F32, tag=f"rcp{g}", name=f"rcp{g}") for g in range(NG)]
            th_last = {}
            pe_g = [pep.tile([128, 512], F32, tag=f"pe{g}", name=f"pe{g}") for g in range(NG)]
            for g in range(NG):
                nc.vector.memset(pe_g[g][:], 0.0)

            def strip_slice(pi, w):
                g, strip = pi // 4, 32 * (pi % 4)
                return pe_g[g][strip:strip + 1, 0:w], (0, strip)

            # ---- main loop over h-chunks ------------------------------
            for hc in range(HC):
                # Wh chain for this h-chunk (bf16), bias = pw + (W_b+U_b)
                pw = pwp.tile([128, BL], F32, tag="pw")
                for kc in range(KC):
                    nc.tensor.matmul(
                        pw[:],
                        lhsT=ww_sb[hc][:, kc * 128:(kc + 1) * 128],
                        rhs=hid_sb[:, kc * BL:(kc + 1) * BL],
                        start=(kc == 0),
                        stop=(kc == KC - 1),
                    )
                nc.vector.tensor_tensor(
                    bias_sb[:, hc * BL:(hc + 1) * BL], pw[:],
                    bc_sb[:, hc:hc + 1].to_broadcast([128, BL]),
                    mybir.AluOpType.add,
                )

                # main fp8 DoubleRow GEMM, ec2-major so each weight pair
                # feeds all bins back-to-back
                uw3 = uw_sb[hc][:].rearrange("p (e two v) -> p e two v", two=2, v=128)
                psb = [pup.tile([128, bw], F32, tag="pu", name=f"pu{t}") for t, (_, bw, _) in enumerate(bins)]
                for ec2 in range(EC2):
                    w3 = uw3[:, ec2]
                    for t, (boff, bw, _) in enumerate(bins):
                        e3 = enc_sb[ec2][:, boff16[t]:boff16[t + 1]].rearrange(
                            "p (two x) -> p two x", two=2)
                        nc.tensor.matmul(
                            psb[t][:],
                            lhsT=w3,
                            rhs=e3[:, :, 0:bw],
                            start=(ec2 == 0),
                            stop=(ec2 == EC2 - 1),
                            perf_mode=DR,
                        )
                # tanh (+ per-position Wh bias), then v-weight into acc
                vcol = vt_sb[:, hc:hc + 1]
                for t, (boff, bw, segs) in enumerate(bins):
                    th = thp.tile([128, bw], BF, tag="th")
                    for pi, lo, w in segs:
                        # hidT columns are stored in placement order, so the
                        # Wh bias column for this segment is pi itself
                        nc.scalar.activation(
                            th[:, lo:lo + w], psb[t][:, lo:lo + w], AF.Tanh,
                            bias=bias_sb[:, hc * BL + pi:hc * BL + pi + 1],
                            scale=1.0 / U_SCALE,
                        )
                    # h-chunks 0..HC-3 accumulate v*tanh on ACT+DVE; HC-3
                    # closes the bf16 partial sum; the last two h-chunks go
                    # straight to PSUM via PE v-matmuls (keeps the tail free
                    # of the ACT/DVE pipeline drain)
                    if hc == 0:
                        nc.scalar.mul(acc[:, boff:boff + bw], th[:], vcol)
                    elif hc < HC - 3:
                        tmp = tmpp.tile([128, bw], F32, tag="tmp")
                        nc.scalar.mul(tmp[:], th[:], vcol)
                        nc.vector.tensor_add(
                            acc[:, boff:boff + bw], acc[:, boff:boff + bw], tmp[:])
                    elif hc == HC - 3:
                        tmp = tmpp.tile([128, bw], F32, tag="tmp")
                        nc.scalar.mul(tmp[:], th[:], vcol)
                        # bf16 partial sum for the partition-reduce matmul
                        nc.vector.tensor_add(
                            accb[:, boff:boff + bw], acc[:, boff:boff + bw], tmp[:])
                        # fold the -1e10 mask row into partition 0
                        nc.vector.tensor_add(
                            accb[0:1, boff:boff + bw],
                            accb[0:1, boff:boff + bw],
                            am_sb[0:1, boff:boff + bw])
                    else:
                        th_last[(hc, t)] = th
                # after a bin's tanh at the last two h-chunks, fold its
                # v-contribution straight into the PSUM energy strips
                if hc >= HC - 2:
                    for t, (boff, bw, segs) in enumerate(bins):
                        if hc == HC - 2 and t == 0:
                            # open each strip's accumulation group with the
                            # bf16 partial-sum reduce (all positions)
                            for pj in range(NPOS):
                                off2, w2 = pl_off[pj]
                                psl, tp = strip_slice(pj, w2)
                                nc.tensor.matmul(
                                    psl, lhsT=ones_c[:, 0:1],
                                    rhs=accb[:, off2:off2 + w2],
                                    start=True, stop=False, tile_position=tp,
                                )
                        th = th_last[(hc, t)]
                        for pi, lo, w in segs:
                            psl, tp = strip_slice(pi, w)
                            nc.tensor.matmul(
                                psl, lhsT=vtb_sb[:, hc:hc + 1],
                                rhs=th[:, lo:lo + w],
                                start=False, stop=(hc == HC - 1),
                                tile_position=tp,
                            )

            # ---- softmax ------------------------------------------------
            for g in range(NG):
                # one bank-wide exp covers all 4 strips; non-strip rows are
                # zero (memset) so they contribute harmless exp(0) garbage
                nc.scalar.activation(
                    res_g[g][:], pe_g[g][:], AF.Exp,
                    accum_out=ss_g[g][:],
                )
                # pad columns [w, 512) of each strip are exactly 0 in PSUM,
                # contributing exp(0)=1 each; subtract that known constant
                nc.vector.tensor_tensor(
                    ss_g[g][:], ss_g[g][:], corr_sb[:, g:g + 1],
                    mybir.AluOpType.subtract)
                nc.vector.reciprocal(rcp_g[g][:], ss_g[g][:])
                nc.vector.tensor_tensor(
                    res_g[g][:], res_g[g][:],
                    rcp_g[g][:, 0:1].to_broadcast([128, 512]),
                    mybir.AluOpType.mult,
                )
                rows = min(4, NPOS - 4 * g)
                src4 = res_g[g][:].rearrange("(f r) x -> f r x", f=4)[0:rows, 0, :]
                eng = (nc.sync, nc.scalar)[g % 2]
                eng.dma_start(out_d[4 * g:4 * g + rows, :], src4)

    nc.finalize()
    return nc


def _prep_inputs(hidden, encoder_outputs, mask, W_w, W_b, U_w, U_b, v_w):
    counts = (~mask).sum(axis=1)
    order = np.argsort(counts, kind="stable")
    # position p on core c holds row order[NCORES*p + c]
    rows = order.reshape(BL, NCORES)
    widths = [max(16, int(counts[rows[p]].max())) for p in range(BL)]
    widths = tuple(widths)
    assert all(w <= 512 for w in widths)

    placements, bins = _plan(widths)
    X = sum(widths)
    XP = -(-X // 16) * 16
    pl_off = [None] * BL
    for boff, bw, segs in bins:
        for pi, lo, w in segs:
            pl_off[pi] = (boff + lo, w)
    # original position -> placement idx
    pos2pl = [None] * BL
    for pi, p in enumerate(placements):
        pos2pl[p] = pi

    # ---- replicated weights ----
    U8 = np.ascontiguousarray((U_w * U_SCALE).T).astype(fp8)       # [E, H]
    uw8_np = np.ascontiguousarray(
        U8.reshape(EC2, 2, 128, HC, 128).transpose(3, 2, 0, 1, 4)
    ).reshape(HC, 128, EC2 * 2 * 128)
    wwT_np = np.ascontiguousarray(W_w.T).astype(bf16)
    wwT_np = np.ascontiguousarray(
        wwT_np.reshape(KC, 128, HC, 128).transpose(2, 1, 0, 3)
    ).reshape(HC, 128, KC * 128)
    vt_np = np.ascontiguousarray(v_w[0].reshape(HC, 128).T).astype(np.float32)
    bc_np = np.ascontiguousarray((W_b + U_b).reshape(HC, 128).T).astype(np.float32)

    enc8_full = encoder_outputs.astype(fp8)                        # [S, B, E]

    NG = (BL + 3) // 4
    corr_np = np.zeros((128, NG), np.float32)
    for pi in range(BL):
        _, w = pl_off[pi]
        corr_np[32 * (pi % 4), pi // 4] = 512 - w

    idx_all = [np.nonzero(~mask[i])[0] for i in range(B)]
    # per-bin 16-padded plane strides (must match _build_nc)
    bw16 = [-(-bw // 16) * 16 for (_, bw, _) in bins]
    boff16 = [0]
    for b in bw16:
        boff16.append(boff16[-1] + 2 * b)
    SUMW = boff16[-1]
    # placement -> (bin idx, local offset, width)
    pl_bin = [None] * BL
    for t, (_, _, segs) in enumerate(bins):
        for pi, lo, w in segs:
            pl_bin[pi] = (t, lo, w)

    in_maps = []
    for c in range(NCORES):
        enc_p = np.zeros((EC2, 128, SUMW), fp8)
        am_p = np.full((XP,), NEG, np.float32)
        hid_rows = np.empty((BL, H), np.float32)
        for p in range(BL):
            r = int(rows[p, c])
            pi = pos2pl[p]
            off, w = pl_off[pi]
            t, lo, _ = pl_bin[pi]
            ix = idx_all[r]
            cnt = len(ix)
            if cnt:
                # [cnt, E] -> [EC2, 2, 128, cnt] -> two planes per bin chunk
                a = enc8_full[ix, r, :].T.reshape(EC2, 2, 128, cnt).transpose(0, 2, 1, 3)
                base = boff16[t]
                enc_p[:, :, base + lo:base + lo + cnt] = a[:, :, 0]
                enc_p[:, :, base + bw16[t] + lo:base + bw16[t] + lo + cnt] = a[:, :, 1]
                am_p[off:off + cnt] = 0.0
            hid_rows[pi] = hidden[r]
        hidT_c = np.ascontiguousarray(
            hid_rows.T.astype(bf16).reshape(KC, 128, BL).transpose(1, 0, 2)
        ).reshape(128, KC * BL)
        in_maps.append({
            "enc8": enc_p,
            "uw8": uw8_np,
            "wwT": wwT_np,
            "hidT": hidT_c,
            "vt": vt_np,
            "bc": bc_np,
            "amask": am_p.astype(bf16).reshape(1, XP),
            "corr": corr_np,
        })
    return in_maps, widths, rows, pos2pl, idx_all


def _run(in_maps, widths, trace=False):
    from concourse import bass_utils
    if widths not in _CACHE:
        _CACHE[widths] = _build_nc(widths)
    nc = _CACHE[widths]
    return bass_utils.run_bass_kernel_spmd(
        nc, in_maps, core_ids=list(range(NCORES)), trace=trace
    )


def kernel(hidden, encoder_outputs, mask, W_w, W_b, U_w, U_b, v_w,
           _trace=False, _return_bkr=False):
    hidden = np.asarray(hidden, dtype=np.float32)
    encoder_outputs = np.asarray(encoder_outputs, dtype=np.float32)
    mask = np.asarray(mask).astype(bool)
    W_w = np.asarray(W_w, dtype=np.float32)
    W_b = np.asarray(W_b, dtype=np.float32)
    U_w = np.asarray(U_w, dtype=np.float32)
    U_b = np.asarray(U_b, dtype=np.float32)
    v_w = np.asarray(v_w, dtype=np.float32)

    in_maps, widths, rows, pos2pl, idx_all = _prep_inputs(
        hidden, encoder_outputs, mask, W_w, W_b, U_w, U_b, v_w)
    bkr = _run(in_maps, widths, trace=_trace)

    out = np.zeros((B, S), np.float32)
    for c in range(NCORES):
        dev = bkr.results[c]["out"]                  # [BL, 512]
        for p in range(BL):
            r = int(rows[p, c])
            ix = idx_all[r]
            cnt = len(ix)
            if cnt:
                out[r, ix] = dev[pos2pl[p], :cnt]
            else:
                out[r, :] = np.float32(1.0 / S)
    if _return_bkr:
        return out, bkr
    return out


# revision 19
# speedup vs baseline: 1.2654x; 1.2654x over previous
"""Bahdanau-attention kernel for Trainium2 (8 NeuronCores, Bass/Tile).

Computation (reference, fp32):
    Wh  = hidden @ W_w.T + W_b                      # [B, H]
    Ue  = einsum('bse,he->bsh', enc^T, U_w) + U_b   # [B, S, H]
    en  = tanh(Wh[:,None,:] + Ue) @ v_w[0]          # [B, S]
    out = softmax(where(mask, -1e10, en), axis=1)

Strategy (v2 - fp8 DoubleRow):
- Data-parallel over batch: 8 rows per core. Rows are rank-sorted by
  unmasked count so position p on every core is padded to the same
  compile-time width w[p]; only unmasked s-columns are packed (exact:
  exp(-1e10) = 0).
- Main GEMM in fp8 e4m3 with perf_mode=DoubleRow: each matmul contracts
  256 rows (two 128-chunks), halving PE streaming time vs bf16. U is
  scaled by 64 before quantization; the tanh activation un-scales.
- Positions are bin-packed into column tiles <= 512 (one PSUM bank
  each). Loop order per h-chunk is ec2-major: one weight pair feeds all
  bins back-to-back so LDWEIGHTS overlaps the matmul stream.
- Wh (bf16) runs as small interleaved chains; W_b+U_b folded into the
  per-partition tanh bias.
- v-weighting runs on the Vector engine (tmp = tanh*v in bf16; acc +=
  tmp in fp32; the last h-chunk's add writes a bf16 copy and the -1e10
  mask row is added into partition 127); final partition reduce is one
  bf16 ones-matmul per position into PSUM strips (partition 32*(i%4)).
- Softmax: one bank-wide Exp (+per-partition accum) per group of 4
  positions, reciprocal, one broadcast multiply, one 4-row output DMA.
- DMA: both HWDGE queues (sync + scalar) stream enc/uw with e0/uw0
  first; gpsimd SWDGE carries the later-needed ww/uw chunks.
"""

import numpy as np
import ml_dtypes

B, S, H, E = 64, 512, 1024, 2048
NCORES = 8
BL = B // NCORES          # rows (positions) per core
HC = H // 128             # h chunks
EC2 = E // 256            # fp8 DoubleRow chunk pairs
KC = H // 128             # k chunks for the Wh matmul
NEG = np.float32(-1e10)
U_SCALE = 64.0

bf16 = ml_dtypes.bfloat16
fp8 = ml_dtypes.float8_e4m3

_CACHE = {}


def _plan(widths):
    """Bin-pack position widths into column tiles <= 512 (first-fit dec.).

    Returns (placements, bins) where placements is a list of original
    position indices in column order and bins is a list of
    (bin_off, bin_w, [(placement_idx, local_off, w), ...]).
    """
    order = sorted(range(len(widths)), key=lambda p: -widths[p])
    bins = []  # list of lists of original position idx
    for p in order:
        for bn in bins:
            if sum(widths[q] for q in bn) + widths[p] <= 512:
                bn.append(p)
                break
        else:
            bins.append([p])
    placements = [p for bn in bins for p in bn]
    out_bins = []
    off = 0
    pi = 0
    for bn in bins:
        lo = 0
        segs = []
        for p in bn:
            segs.append((pi, lo, widths[p]))
            lo += widths[p]
            pi += 1
        out_bins.append((off, lo, segs))
        off += lo
    return placements, out_bins


def _build_nc(widths):
    """Per-core program; widths = per-position packed col counts (<=512)."""
    import concourse.mybir as mybir
    import concourse.tile as tile
    from concourse import bacc

    F32 = mybir.dt.float32
    F32R = mybir.dt.float32r
    BF = mybir.dt.bfloat16
    F8 = mybir.dt.float8e4
    AF = mybir.ActivationFunctionType
    DR = mybir.MatmulPerfMode.DoubleRow

    placements, bins = _plan(widths)
    NPOS = len(widths)
    X = sum(widths)
    XP = -(-X // 16) * 16           # plane stride must be 16B-aligned
    NB = len(bins)
    assert NB + 3 <= 8, f"too many PSUM banks: {NB}"
    # placement -> (col_off, width)
    pl_off = [None] * NPOS
    for boff, bw, segs in bins:
        for pi, lo, w in segs:
            pl_off[pi] = (boff + lo, w)

    # per-bin 16-padded plane stride for the fp8 DoubleRow rhs
    bw16 = [-(-bw // 16) * 16 for (_, bw, _) in bins]
    boff16 = [0]
    for b in bw16:
        boff16.append(boff16[-1] + 2 * b)
    SUMW = boff16[-1]

    nc = bacc.Bacc(num_swdge_queues=4)
    enc8 = nc.declare_dram_parameter("enc8", [EC2, 128, SUMW], F8, isOutput=False)
    uw8 = nc.declare_dram_parameter("uw8", [HC, 128, EC2 * 2 * 128], F8, isOutput=False)
    wwT = nc.declare_dram_parameter("wwT", [HC, 128, KC * 128], BF, isOutput=False)
    hidT = nc.declare_dram_parameter("hidT", [128, KC * BL], BF, isOutput=False)
    vt = nc.declare_dram_parameter("vt", [128, HC], F32, isOutput=False)
    bc = nc.declare_dram_parameter("bc", [128, HC], F32, isOutput=False)
    amask = nc.declare_dram_parameter("amask", [1, XP], BF, isOutput=False)
    NG = (NPOS + 3) // 4
    corr = nc.declare_dram_parameter("corr", [128, NG], F32, isOutput=False)
    out_d = nc.declare_dram_parameter("out", [NPOS, 512], F32, isOutput=True)

    with tile.TileContext(nc) as tc:
        with (
            tc.tile_pool(name="const", bufs=1) as cst,
            tc.tile_pool(name="wpool", bufs=1) as wp,
            tc.tile_pool(name="encp", bufs=EC2) as encp,
            tc.tile_pool(name="thp", bufs=4) as thp,
            tc.tile_pool(name="tmpp", bufs=3) as tmpp,
            tc.tile_pool(name="pwp", bufs=1, space="PSUM") as pwp,
            tc.tile_pool(name="pup", bufs=NB, space="PSUM") as pup,
            tc.tile_pool(name="pep", bufs=1, space="PSUM") as pep,
        ):
            # ---- DMAs: sync + scalar are HWDGE queues (fast); gpsimd is
            # software DGE and carries only later-needed weights/constants.
            hid_sb = cst.tile([128, KC * BL], BF, tag="hid")
            nc.scalar.dma_start(hid_sb[:], hidT[:])
            bc_sb = cst.tile([128, HC], F32, tag="bc")
            nc.gpsimd.dma_start(bc_sb[:], bc[:])
            vt_sb = cst.tile([128, HC], F32, tag="vt")
            nc.gpsimd.dma_start(vt_sb[:], vt[:])
            am_sb = cst.tile([1, XP], BF, tag="am")
            nc.gpsimd.dma_start(am_sb[:], amask[:])

            ww_sb = [wp.tile([128, KC * 128], BF, tag=f"ww{h}", name=f"ww{h}") for h in range(HC)]
            uw_sb = [wp.tile([128, EC2 * 256], F8, tag=f"uw{h}", name=f"uw{h}") for h in range(HC)]
            enc_sb = [encp.tile([128, SUMW], F8, tag="enc", name=f"enc{k}") for k in range(EC2)]

            def dma2(dst, src, width, e1, e2):
                h = (width // 2) & ~3
                e1.dma_start(dst[:, 0:h], src[:, 0:h])
                e2.dma_start(dst[:, h:width], src[:, h:width])

            # startup-critical: small chunks so the Wh chain and first main
            # matmuls can start as data lands; 3-way split incl. gpsimd
            for c in range(4):
                eng = (nc.sync, nc.scalar)[c % 2]
                eng.dma_start(ww_sb[0][:, c * 256:(c + 1) * 256],
                              wwT[0][:, c * 256:(c + 1) * 256])
            third = (EC2 * 256 // 3) & ~15
            nc.sync.dma_start(uw_sb[0][:, 0:third], uw8[0][:, 0:third])
            nc.scalar.dma_start(uw_sb[0][:, third:2 * third], uw8[0][:, third:2 * third])
            nc.gpsimd.dma_start(uw_sb[0][:, 2 * third:], uw8[0][:, 2 * third:])
            # e0/e1 in 3-way fine chunks for earliest start; e2..e5 split
            # across sync/scalar/gpsimd thirds (trigger cost ~0.7us each)
            qi = 0
            for k in (0, 1):
                for t in range(NB):
                    lo, hi = boff16[t], boff16[t + 1]
                    eng = (nc.sync, nc.scalar, nc.gpsimd)[qi % 3]
                    qi += 1
                    eng.dma_start(enc_sb[k][:, lo:hi], enc8[k][:, lo:hi])
            third = (SUMW // 3) & ~15
            for k in range(2, 6):
                nc.sync.dma_start(enc_sb[k][:, 0:third], enc8[k][:, 0:third])
                nc.scalar.dma_start(enc_sb[k][:, third:2 * third],
                                    enc8[k][:, third:2 * third])
                nc.gpsimd.dma_start(enc_sb[k][:, 2 * third:SUMW],
                                    enc8[k][:, 2 * third:SUMW])
                if k == 2:
                    nc.scalar.dma_start(uw_sb[1][:], uw8[1][:])
                if k == 4:
                    nc.sync.dma_start(uw_sb[2][:], uw8[2][:])
                if k == 5:
                    nc.scalar.dma_start(uw_sb[3][:], uw8[3][:])
            # e6/e7 + later weights ride the SWDGE queue, relieving the
            # HWDGE pair during the startup-critical window
            nc.gpsimd.dma_start(ww_sb[1][:], wwT[1][:])
            for k in (6, 7):
                h = (SUMW // 2) & ~15
                nc.gpsimd.dma_start(enc_sb[k][:, 0:h], enc8[k][:, 0:h])
                nc.gpsimd.dma_start(enc_sb[k][:, h:SUMW], enc8[k][:, h:SUMW])
            for k in range(2, HC):
                nc.gpsimd.dma_start(ww_sb[k][:], wwT[k][:])
            for h in range(4, HC):
                nc.gpsimd.dma_start(uw_sb[h][:], uw8[h][:])

            bias_sb = cst.tile([128, HC * BL], F32, tag="bias")
            vtb_sb = cst.tile([128, HC], BF, tag="vtb")
            nc.vector.tensor_copy(vtb_sb[:], vt_sb[:])
            acc = cst.tile([128, XP], F32, tag="acc")
            accb = cst.tile([128, XP], BF, tag="accb")
            ones_c = cst.tile([128, 1], BF, tag="onesc")
            nc.vector.memset(ones_c[:], 1.0)

            corr_sb = cst.tile([128, NG], F32, tag="corr")
            nc.gpsimd.dma_start(corr_sb[:], corr[:])
            res_g = [cst.tile([128, 512], F32, tag=f"res{g}", name=f"res{g}") for g in range(NG)]
            ss_g = [cst.tile([128, 1], F32, tag=f"ss{g}", name=f"ss{g}") for g in range(NG)]
            rcp_g = [cst.tile([128, 1], F32, tag=f"rcp{g}", name=f"rcp{g}") for g in range(NG)]
            th_last = {}
            pe_g = [pep.tile([128, 512], F32, tag=f"pe{g}", name=f"pe{g}") for g in range(NG)]
            for g in range(NG):
                nc.vector.memset(pe_g[g][:], 0.0)

            def strip_slice(pi, w):
                g, strip = pi // 4, 32 * (pi % 4)
                return pe_g[g][strip:strip + 1, 0:w], (0, strip)

            # ---- main loop over h-chunks ------------------------------
            for hc in range(HC):
                # Wh chain for this h-chunk (bf16), bias = pw + (W_b+U_b)
                pw = pwp.tile([128, BL], F32, tag="pw")
                for kc in range(KC):
                    nc.tensor.matmul(
                        pw[:],
                        lhsT=ww_sb[hc][:, kc * 128:(kc + 1) * 128],
                        rhs=hid_sb[:, kc * BL:(kc + 1) * BL],
                        start=(kc == 0),
                        stop=(kc == KC - 1),
                    )
                nc.vector.tensor_tensor(
                    bias_sb[:, hc * BL:(hc + 1) * BL], pw[:],
                    bc_sb[:, hc:hc + 1].to_broadcast([128, BL]),
                    mybir.AluOpType.add,
                )

                # main fp8 DoubleRow GEMM, ec2-major so each weight pair
                # feeds all bins back-to-back
                uw3 = uw_sb[hc][:].rearrange("p (e two v) -> p e two v", two=2, v=128)
                psb = [pup.tile([128, bw], F32, tag="pu", name=f"pu{t}") for t, (_, bw, _) in enumerate(bins)]
                for ec2 in range(EC2):
                    w3 = uw3[:, ec2]
                    for t, (boff, bw, _) in enumerate(bins):
                        e3 = enc_sb[ec2][:, boff16[t]:boff16[t + 1]].rearrange(
                            "p (two x) -> p two x", two=2)
                        nc.tensor.matmul(
                            psb[t][:],
                            lhsT=w3,
                            rhs=e3[:, :, 0:bw],
                            start=(ec2 == 0),
                            stop=(ec2 == EC2 - 1),
                            perf_mode=DR,
                        )
                # tanh (+ per-position Wh bias), then v-weight into acc
                vcol = vt_sb[:, hc:hc + 1]
                for t, (boff, bw, segs) in enumerate(bins):
                    th = thp.tile([128, bw], BF, tag="th")
                    for pi, lo, w in segs:
                        # hidT columns are stored in placement order, so the
                        # Wh bias column for this segment is pi itself
                        nc.scalar.activation(
                            th[:, lo:lo + w], psb[t][:, lo:lo + w], AF.Tanh,
                            bias=bias_sb[:, hc * BL + pi:hc * BL + pi + 1],
                            scale=1.0 / U_SCALE,
                        )
                    # h-chunks 0..HC-3 accumulate v*tanh on ACT+DVE; HC-3
                    # closes the bf16 partial sum; the last two h-chunks go
                    # straight to PSUM via PE v-matmuls (keeps the tail free
                    # of the ACT/DVE pipeline drain)
                    if hc == 0:
                        nc.scalar.mul(acc[:, boff:boff + bw], th[:], vcol)
                    elif hc < HC - 3:
                        tmp = tmpp.tile([128, bw], F32, tag="tmp")
                        nc.scalar.mul(tmp[:], th[:], vcol)
                        nc.vector.tensor_add(
                            acc[:, boff:boff + bw], acc[:, boff:boff + bw], tmp[:])
                    elif hc == HC - 3:
                        tmp = tmpp.tile([128, bw], F32, tag="tmp")
                        nc.scalar.mul(tmp[:], th[:], vcol)
                        # bf16 partial sum for the partition-reduce matmul
                        nc.vector.tensor_add(
                            accb[:, boff:boff + bw], acc[:, boff:boff + bw], tmp[:])
                        # fold the -1e10 mask row into partition 0
                        nc.vector.tensor_add(
                            accb[0:1, boff:boff + bw],
                            accb[0:1, boff:boff + bw],
                            am_sb[0:1, boff:boff + bw])
                    else:
                        th_last[(hc, t)] = th
                # after a bin's tanh at the last two h-chunks, fold its
                # v-contribution straight into the PSUM energy strips
                if hc >= HC - 2:
                    for t, (boff, bw, segs) in enumerate(bins):
                        if hc == HC - 2 and t == 0:
                            # open each strip's accumulation group with the
                            # bf16 partial-sum reduce (all positions)
                            for pj in range(NPOS):
                                off2, w2 = pl_off[pj]
                                psl, tp = strip_slice(pj, w2)
                                nc.tensor.matmul(
                                    psl, lhsT=ones_c[:, 0:1],
                                    rhs=accb[:, off2:off2 + w2],
                                    start=True, stop=False, tile_position=tp,
                                )
                        th = th_last[(hc, t)]
                        for pi, lo, w in segs:
                            psl, tp = strip_slice(pi, w)
                            nc.tensor.matmul(
                                psl, lhsT=vtb_sb[:, hc:hc + 1],
                                rhs=th[:, lo:lo + w],
                                start=False, stop=(hc == HC - 1),
                                tile_position=tp,
                            )

            # ---- softmax ------------------------------------------------
            for g in range(NG):
                # one bank-wide exp covers all 4 strips; non-strip rows are
                # zero (memset) so they contribute harmless exp(0) garbage
                nc.scalar.activation(
                    res_g[g][:], pe_g[g][:], AF.Exp,
                    accum_out=ss_g[g][:],
                )
                # pad columns [w, 512) of each strip are exactly 0 in PSUM,
                # contributing exp(0)=1 each; subtract that known constant
                nc.vector.tensor_tensor(
                    ss_g[g][:], ss_g[g][:], corr_sb[:, g:g + 1],
                    mybir.AluOpType.subtract)
                nc.vector.reciprocal(rcp_g[g][:], ss_g[g][:])
                nc.vector.tensor_tensor(
                    res_g[g][:], res_g[g][:],
                    rcp_g[g][:, 0:1].to_broadcast([128, 512]),
                    mybir.AluOpType.mult,
                )
                rows = min(4, NPOS - 4 * g)
                src4 = res_g[g][:].rearrange("(f r) x -> f r x", f=4)[0:rows, 0, :]
                eng = (nc.sync, nc.scalar)[g % 2]
                eng.dma_start(out_d[4 * g:4 * g + rows, :], src4)

    nc.finalize()
    return nc


def _prep_inputs(hidden, encoder_outputs, mask, W_w, W_b, U_w, U_b, v_w):
    counts = (~mask).sum(axis=1)
    order = np.argsort(counts, kind="stable")
    # position p on core c holds row order[NCORES*p + c]
    rows = order.reshape(BL, NCORES)
    widths = [max(16, int(counts[rows[p]].max())) for p in range(BL)]
    widths = tuple(widths)
    assert all(w <= 512 for w in widths)

    placements, bins = _plan(widths)
    X = sum(widths)
    XP = -(-X // 16) * 16
    pl_off = [None] * BL
    for boff, bw, segs in bins:
        for pi, lo, w in segs:
            pl_off[pi] = (boff + lo, w)
    # original position -> placement idx
    pos2pl = [None] * BL
    for pi, p in enumerate(placements):
        pos2pl[p] = pi

    # ---- replicated weights ----
    U8 = np.ascontiguousarray((U_w * U_SCALE).T).astype(fp8)       # [E, H]
    uw8_np = np.ascontiguousarray(
        U8.reshape(EC2, 2, 128, HC, 128).transpose(3, 2, 0, 1, 4)
    ).reshape(HC, 128, EC2 * 2 * 128)
    wwT_np = np.ascontiguousarray(W_w.T).astype(bf16)
    wwT_np = np.ascontiguousarray(
        wwT_np.reshape(KC, 128, HC, 128).transpose(2, 1, 0, 3)
    ).reshape(HC, 128, KC * 128)
    vt_np = np.ascontiguousarray(v_w[0].reshape(HC, 128).T).astype(np.float32)
    bc_np = np.ascontiguousarray((W_b + U_b).reshape(HC, 128).T).astype(np.float32)

    enc8_full = encoder_outputs.astype(fp8)                        # [S, B, E]

    NG = (BL + 3) // 4
    corr_np = np.zeros((128, NG), np.float32)
    for pi in range(BL):
        _, w = pl_off[pi]
        corr_np[32 * (pi % 4), pi // 4] = 512 - w

    idx_all = [np.nonzero(~mask[i])[0] for i in range(B)]
    # per-bin 16-padded plane strides (must match _build_nc)
    bw16 = [-(-bw // 16) * 16 for (_, bw, _) in bins]
    boff16 = [0]
    for b in bw16:
        boff16.append(boff16[-1] + 2 * b)
    SUMW = boff16[-1]
    # placement -> (bin idx, local offset, width)
    pl_bin = [None] * BL
    for t, (_, _, segs) in enumerate(bins):
        for pi, lo, w in segs:
            pl_bin[pi] = (t, lo, w)

    in_maps = []
    for c in range(NCORES):
        enc_p = np.zeros((EC2, 128, SUMW), fp8)
        am_p = np.full((XP,), NEG, np.float32)
        hid_rows = np.empty((BL, H), np.float32)
        for p in range(BL):
            r = int(rows[p, c])
            pi = pos2pl[p]
            off, w = pl_off[pi]
            t, lo, _ = pl_bin[pi]
            ix = idx_all[r]
            cnt = len(ix)
            if cnt:
                # [cnt, E] -> [EC2, 2, 128, cnt] -> two planes per bin chunk
                a = enc8_full[ix, r, :].T.reshape(EC2, 2, 128, cnt).transpose(0, 2, 1, 3)
                base = boff16[t]
                enc_p[:, :, base + lo:base + lo + cnt] = a[:, :, 0]
                enc_p[:, :, base + bw16[t] + lo:base + bw16[t] + lo + cnt] = a[:, :, 1]
                am_p[off:off + cnt] = 0.0
            hid_rows[pi] = hidden[r]
        hidT_c = np.ascontiguousarray(
            hid_rows.T.astype(bf16).reshape(KC, 128, BL).transpose(1, 0, 2)
        ).reshape(128, KC * BL)
        in_maps.append({
            "enc8": enc_p,
            "uw8": uw8_np,
            "wwT": wwT_np,
            "hidT": hidT_c,
            "vt": vt_np,
            "bc": bc_np,
            "amask": am_p.astype(bf16).reshape(1, XP),
            "corr": corr_np,
        })
    return in_maps, widths, rows, pos2pl, idx_all


def _run(in_maps, widths, trace=False):
    from concourse import bass_utils
    if widths not in _CACHE:
        _CACHE[widths] = _build_nc(widths)
    nc = _CACHE[widths]
    return bass_utils.run_bass_kernel_spmd(
        nc, in_maps, core_ids=list(range(NCORES)), trace=trace
    )


def kernel(hidden, encoder_outputs, mask, W_w, W_b, U_w, U_b, v_w,
           _trace=False, _return_bkr=False):
    hidden = np.asarray(hidden, dtype=np.float32)
    encoder_outputs = np.asarray(encoder_outputs, dtype=np.float32)
    mask = np.asarray(mask).astype(bool)
    W_w = np.asarray(W_w, dtype=np.float32)
    W_b = np.asarray(W_b, dtype=np.float32)
    U_w = np.asarray(U_w, dtype=np.float32)
    U_b = np.asarray(U_b, dtype=np.float32)
    v_w = np.asarray(v_w, dtype=np.float32)

    in_maps, widths, rows, pos2pl, idx_all = _prep_inputs(
        hidden, encoder_outputs, mask, W_w, W_b, U_w, U_b, v_w)
    bkr = _run(in_maps, widths, trace=_trace)

    out = np.zeros((B, S), np.float32)
    for c in range(NCORES):
        dev = bkr.results[c]["out"]                  # [BL, 512]
        for p in range(BL):
            r = int(rows[p, c])
            ix = idx_all[r]
            cnt = len(ix)
            if cnt:
                out[r, ix] = dev[pos2pl[p], :cnt]
            else:
                out[r, :] = np.float32(1.0 / S)
    if _return_bkr:
        return out, bkr
    return out
